# revision 1
# baseline (speedup 1.0000x reference)
"""Custom GRU cell kernel for Trainium2, data-parallel over batch on 8 NeuronCores.

Layout strategy: everything on-device lives in [feature=128 partitions, batch free]
("transposed") layout so the six 128x128 weight matrices are the stationary matmul
operands and no on-device transposes are needed. The host pre-transposes x/h0 and
post-transposes the output history.

Per-step dataflow (per core, B_local=256, all tiles [128, 256] unless noted):
  PE : ps_rz[:,0:256]  = W_r.T x_t ; += U_r.T h     (one PSUM bank, [128,512])
       ps_rz[:,256:512]= W_z.T x_t ; += U_z.T h
       ps_xh = W_h.T x_t            (accumulation group left open)
       ps_mmh= U_h.T h
       ps_xh += I.T @ t1            (identity matmul folds the r*(U_h h) add)
  ACT: ru   = sigmoid(ps_rz)        ([128,512], r and u in one op)
       htil = tanh(ps_xh + b_h)
  DVE: t1 = ps_mmh * r ; g = u * dif ; e = g * a_bc ; h' = h + e
  GPS: dif = htil - h ; a_bc = partition_broadcast(a chunk) once per chunk
State h is bf16 and h' is written straight into the output chunk, DMA'd out in
[U, T_chunk, B_local] layout; matmul inputs (x, h, weights) are bf16, PSUM is f32.
"""

import sys

sys.path.insert(0, "/opt/trn_rl_repo")

import numpy as np
import ml_dtypes

import concourse.bass as bass  # noqa: F401  (import registers rust bindings)
import concourse.mybir as mybir
import concourse.tile as tile
from concourse import bacc
from concourse.bass_utils import run_bass_kernel_spmd

BF16 = mybir.dt.bfloat16
F32 = mybir.dt.float32
AF = mybir.ActivationFunctionType
OP = mybir.AluOpType

B, T, U = 2048, 200, 128
NCORES = 8
BL = B // NCORES  # 256 batch rows per core
TC = 25  # timesteps per chunk
NCHUNK = T // TC

# knobs (flipped during tuning)
USE_GPS_BCAST = False  # a-broadcast via gpsimd.partition_broadcast vs PE K=1 matmul
DIF_ON_GPS = False  # (htil - h) on GPSIMD vs VectorE

# set by test.py to collect profile info; kernel() stores results here
PROFILE = False
LAST_RESULT = None
LAST_IN_MAPS = None

_cache = {}


def _build(has_brz: bool, T_=T, TC_=TC, BL_=BL, reps=1):
    """Build + compile the per-core Bass program. has_brz: b_r/b_z nonzero path."""
    NCHUNK_ = T_ // TC_
    nc = bacc.Bacc("TRN2", target_bir_lowering=False)

    xt = nc.dram_tensor("xt", [U, T_, BL_], BF16, kind="ExternalInput")
    av = nc.dram_tensor("av", [T_ * BL_], BF16, kind="ExternalInput")
    h0t = nc.dram_tensor("h0t", [U, BL_], BF16, kind="ExternalInput")
    wcat = nc.dram_tensor("wcat", [6, U, U], BF16, kind="ExternalInput")
    ident_d = nc.dram_tensor("ident", [U, U], BF16, kind="ExternalInput")
    biases = nc.dram_tensor("biases", [U, 3], F32, kind="ExternalInput")
    ones_d = nc.dram_tensor("ones1", [1, U], BF16, kind="ExternalInput")
    outt = nc.dram_tensor("outt", [U, T_, BL_], BF16, kind="ExternalOutput")

    with tile.TileContext(nc) as tc:
        with (
            tc.tile_pool(name="const", bufs=1) as cpool,
            tc.tile_pool(name="xchunk", bufs=2) as xpool,
            tc.tile_pool(name="achunk", bufs=2) as apool,
            tc.tile_pool(name="abc", bufs=2) as abcpool,
            tc.tile_pool(name="ochunk", bufs=2) as opool,
            tc.tile_pool(name="work", bufs=4) as wpool,
            tc.tile_pool(name="psum", bufs=2, space="PSUM") as ppool,
        ):
            wts = []
            for i in range(6):
                wt = cpool.tile([U, U], BF16, tag=f"w{i}")
                nc.sync.dma_start(wt[:], wcat[i])
                wts.append(wt)
            w_r, u_r, w_z, u_z, w_h, u_h = wts
            ident = cpool.tile([U, U], BF16, tag="ident")
            nc.sync.dma_start(ident[:], ident_d[:])
            ones1 = cpool.tile([1, U], BF16, tag="ones1")
            nc.sync.dma_start(ones1[:], ones_d[:])
            btile = cpool.tile([U, 3], F32, tag="biases")
            nc.sync.dma_start(btile[:], biases[:])
            b_r_ap = btile[:, 0:1]
            b_z_ap = btile[:, 1:2]
            b_h_ap = btile[:, 2:3]
            h0tile = cpool.tile([U, BL_], BF16, tag="h0")
            nc.sync.dma_start(h0tile[:], h0t[:])

            for _rep in range(reps):
                xchs = {}

                def load_chunk(k):
                    if k >= NCHUNK_ or k in xchs:
                        return
                    t0, t1x = k * TC_, (k + 1) * TC_
                    xch = xpool.tile([U, TC_, BL_], BF16, tag="xch")
                    nc.sync.dma_start(xch[:], xt[:, t0:t1x, :])
                    ach = apool.tile([1, TC_ * BL_], BF16, tag="ach")
                    nc.sync.dma_start(ach[:], av[t0 * BL_ : t1x * BL_])
                    xchs[k] = (xch, ach)

                def emit_xside(t):
                    """x-dependent matmuls for step t (off the h critical path)."""
                    k, dt = divmod(t, TC_)
                    xch, ach = xchs[k]
                    xs = xch[:, dt, :]
                    ps_rz = ppool.tile([U, 2 * BL_], F32, tag="ps_rz")
                    nc.tensor.matmul(ps_rz[:, 0:BL_], w_r[:], xs, start=True, stop=False)
                    nc.tensor.matmul(ps_rz[:, BL_:], w_z[:], xs, start=False, stop=False)
                    ps_xh_full = ppool.tile([U, 2 * BL_], F32, tag="ps_xh")
                    ps_xh = ps_xh_full[:, 0:BL_]
                    nc.tensor.matmul(ps_xh, w_h[:], xs, start=True, stop=False)
                    ps_a = ppool.tile([U, BL_], F32, tag="ps_a")
                    nc.tensor.matmul(
                        ps_a[:], ones1[:], ach[:, dt * BL_ : (dt + 1) * BL_],
                        start=True, stop=True,
                    )
                    return ps_rz, ps_xh, ps_a

                h_prev = h0tile[:]
                load_chunk(0)
                pending = emit_xside(0)
                och = None
                for t in range(T_):
                    k, dt = divmod(t, TC_)
                    if dt == 0:
                        load_chunk(k + 1)
                        och = opool.tile([U, TC_, BL_], BF16, tag="och")
                    ps_rz, ps_xh, ps_a = pending

                    ps_mmh = ppool.tile([U, BL_], F32, tag="ps_mmh")
                    nc.tensor.matmul(ps_mmh[:], u_h[:], h_prev, start=True, stop=True)
                    nc.tensor.matmul(ps_rz[:, BL_:], u_z[:], h_prev, start=False, stop=False)
                    nc.tensor.matmul(ps_rz[:, 0:BL_], u_r[:], h_prev, start=False, stop=True)

                    r_sb = wpool.tile([U, BL_], BF16, tag="r_sb")
                    if has_brz:
                        nc.scalar.activation(r_sb[:], ps_rz[:, 0:BL_], AF.Sigmoid, bias=b_r_ap)
                    else:
                        nc.scalar.activation(r_sb[:], ps_rz[:, 0:BL_], AF.Sigmoid)
                    u_sb = wpool.tile([U, BL_], BF16, tag="u_sb")
                    if has_brz:
                        nc.scalar.activation(u_sb[:], ps_rz[:, BL_:], AF.Sigmoid, bias=b_z_ap)
                    else:
                        nc.scalar.activation(u_sb[:], ps_rz[:, BL_:], AF.Sigmoid)

                    t1 = wpool.tile([U, BL_], BF16, tag="t1")
                    nc.vector.tensor_tensor(t1[:], ps_mmh[:], r_sb[:], OP.mult)
                    nc.tensor.matmul(ps_xh, ident[:], t1[:], start=False, stop=True)
                    if t + 1 < T_:
                        pending = emit_xside(t + 1)

                    # off-chain while idmm/tanh run: uhat = a*u, m1 = (uhat-1)*h
                    uhat = wpool.tile([U, BL_], BF16, tag="uhat")
                    nc.vector.tensor_tensor(uhat[:], u_sb[:], ps_a[:], OP.mult)
                    m1 = wpool.tile([U, BL_], BF16, tag="m1")
                    nc.vector.scalar_tensor_tensor(
                        m1[:], uhat[:], 1.0, h_prev, OP.subtract, OP.mult
                    )
                    htil = wpool.tile([U, BL_], BF16, tag="htil")
                    nc.scalar.activation(htil[:], ps_xh, AF.Tanh, bias=b_h_ap)
                    # on-chain tail: hn = uhat*htil - (uhat-1)*h
                    m2 = wpool.tile([U, BL_], BF16, tag="m2")
                    nc.vector.tensor_tensor(m2[:], uhat[:], htil[:], OP.mult)
                    hn = och[:, dt, :]
                    nc.vector.tensor_tensor(hn, m2[:], m1[:], OP.subtract)
                    h_prev = hn

                    if dt == TC_ - 1:
                        nc.sync.dma_start(outt[:, k * TC_ : (k + 1) * TC_, :], och[:])
                        xchs.pop(k, None)

    nc.compile()
    return nc


def kernel(inputs, h0, W_r, U_r, b_r, W_z, U_z, b_z, W_h, U_h, b_h):
    global LAST_RESULT
    inputs = np.asarray(inputs, dtype=np.float32)
    h0 = np.asarray(h0, dtype=np.float32)
    ws = [np.asarray(w, dtype=np.float32) for w in (W_r, U_r, W_z, U_z, W_h, U_h)]
    bs = [np.asarray(b, dtype=np.float32) for b in (b_r, b_z, b_h)]

    has_brz = bool(np.any(bs[0]) or np.any(bs[1]))
    key = has_brz
    if key not in _cache:
        _cache[key] = _build(has_brz)
    nc = _cache[key]

    bf = ml_dtypes.bfloat16
    wcat = np.stack([w.astype(bf) for w in ws])  # [6, U, U]
    ident = np.eye(U, dtype=bf)
    ones1 = np.ones((1, U), dtype=bf)
    biases = np.stack([bs[0], bs[1], bs[2]], axis=1).astype(np.float32)  # [U, 3]

    x = inputs[:, :, :U]  # [B, T, U]
    a = inputs[:, :, U]  # [B, T]

    in_maps = []
    for c in range(NCORES):
        sl = slice(c * BL, (c + 1) * BL)
        xt_c = np.ascontiguousarray(x[sl].transpose(2, 1, 0)).astype(bf)  # [U,T,BL]
        a_c = np.ascontiguousarray(a[sl].T).astype(bf).reshape(T * BL)  # [T*BL]
        h0t_c = np.ascontiguousarray(h0[sl].T).astype(bf)  # [U, BL]
        in_maps.append(
            {
                "xt": xt_c,
                "av": a_c,
                "h0t": h0t_c,
                "wcat": wcat,
                "ident": ident,
                "biases": biases,
                "ones1": ones1,
            }
        )

    res = run_bass_kernel_spmd(nc, in_maps, list(range(NCORES)), trace=PROFILE)
    global LAST_IN_MAPS
    LAST_IN_MAPS = in_maps
    LAST_RESULT = res

    out = np.empty((B, T, U), dtype=np.float32)
    for c in range(NCORES):
        sl = slice(c * BL, (c + 1) * BL)
        # outt: [U, T, BL] bf16 -> [BL, T, U] f32
        out[sl] = res.results[c]["outt"].astype(np.float32).transpose(2, 1, 0)
    return out



# revision 2
# speedup vs baseline: 1.3389x; 1.3389x over previous
"""Custom GRU cell kernel v4: two interleaved batch-chains, weight-switch-aware.

HW-calibrated cost model (measured): a matmul whose stationary weight differs
from the previous one pays ~+95ns for the weight load; same-weight is free.
ACT activation = (FD+222)*0.833ns, DVE TT = (FD/2+58)*1.042 (bf16 SBUF) or
(FD+120)*1.042 (PSUM src), Pool TT = ~165ns @ FD=128.

Design:
- chains A (batch cols 0:128) and B (128:256), independent recurrences,
  phases interleaved for latency hiding (as v3).
- x-side projections for both chains done as single FD=256 matmuls writing
  both chains' PSUM banks through one [U, 2, 512] two-bank tile (strided
  output AP) -> 3 weight switches per step instead of 6.
- no identity matmul: s = xh + t1 computed on DVE (PSUM + SBUF add); tanh
  reads s from SBUF. The xh bank closes at x-side time.
- recurrent matmuls per chain FD=128 (3 switches per chain).
- elementwise: DVE does t1, s, m1(STT); GPSIMD does uhat, m2, hn.

PSUM per step: rzm_both [U,2,512] = [r|z|mmh|-]*2 chains (bank c closes at
chain c's uh matmul), xhp [U,2,256] = both xh (closes at wh x-pair).
"""

import sys

sys.path.insert(0, "/opt/trn_rl_repo")

import numpy as np
import ml_dtypes

import concourse.bass as bass  # noqa: F401
import concourse.mybir as mybir
import concourse.tile as tile
from concourse import bacc
from concourse.bass_utils import run_bass_kernel_spmd

BF16 = mybir.dt.bfloat16
F32 = mybir.dt.float32
AF = mybir.ActivationFunctionType
OP = mybir.AluOpType

B, T, U = 2048, 200, 128
NCORES = 8
BL = B // NCORES  # 256
BH = BL // 2  # 128 per chain
TC = 25
NCHUNK = T // TC

PROFILE = False
LAST_RESULT = None
LAST_IN_MAPS = None

_cache = {}


def _build(has_bias: bool, T_=T, TC_=TC, BL_=BL, reps=1):
    BH_ = BL_ // 2
    NCHUNK_ = T_ // TC_
    nc = bacc.Bacc("TRN2", target_bir_lowering=False)

    xt = nc.dram_tensor("xt", [U, T_, BL_], BF16, kind="ExternalInput")
    abc = nc.dram_tensor("abc", [U, T_, BL_], BF16, kind="ExternalInput")
    h0t = nc.dram_tensor("h0t", [U, BL_], BF16, kind="ExternalInput")
    wcat = nc.dram_tensor("wcat", [9, U, U], BF16, kind="ExternalInput")
    ident_d = nc.dram_tensor("ident", [U, U], BF16, kind="ExternalInput")
    brow_d = nc.dram_tensor("brow", [3, U], BF16, kind="ExternalInput")
    outt = nc.dram_tensor("outt", [U, T_, BL_], BF16, kind="ExternalOutput")

    with tile.TileContext(nc) as tc:
        with (
            tc.tile_pool(name="const", bufs=1) as cpool,
            tc.tile_pool(name="xchunk", bufs=2) as xpool,
            tc.tile_pool(name="achunk", bufs=2) as apool,
            tc.tile_pool(name="ochunk", bufs=2) as opool,
            tc.tile_pool(name="work", bufs=3) as wpool,
            tc.tile_pool(name="psum", bufs=2, space="PSUM") as ppool,
        ):
            wts = []
            for i in range(6):
                wt = cpool.tile([U, U], BF16, tag=f"w{i}", name=f"w{i}")
                nc.sync.dma_start(wt[:], wcat[i])
                wts.append(wt)
            w_r, u_r, w_z, u_z, w_h, u_h = wts
            ident = cpool.tile([U, U], BF16, tag="ident")
            nc.sync.dma_start(ident[:], ident_d[:])
            h0tile = cpool.tile([U, BL_], BF16, tag="h0")
            nc.sync.dma_start(h0tile[:], h0t[:])
            if has_bias:
                btile = cpool.tile([3, U], BF16, tag="brow")
                nc.sync.dma_start(btile[:], brow_d[:])
                onesr = cpool.tile([1, 2 * BH_], BF16, tag="onesr")
                nc.vector.memset(onesr[:], 1.0)

            for _rep in range(reps):
                xchs = {}
                ochs = {}
                h_prev = {0: h0tile[:, 0:BH_], 1: h0tile[:, BH_:]}

                def load_chunk(k):
                    if k >= NCHUNK_ or k in xchs:
                        return
                    t0, t1x = k * TC_, (k + 1) * TC_
                    xch = xpool.tile([U, TC_, BL_], BF16, tag="xch", name="xch")
                    nc.sync.dma_start(xch[:], xt[:, t0:t1x, :])
                    ach = apool.tile([U, TC_, BL_], BF16, tag="ach", name="ach")
                    nc.sync.dma_start(ach[:], abc[:, t0:t1x, :])
                    xchs[k] = (xch, ach)

                def xside(t):
                    """x-projections for BOTH chains, step t. Matmuls ordered in
                    same-weight runs (second of each pair reloads no weights).
                    Per-matmul outputs stay within one PSUM bank."""
                    k, dt = divmod(t, TC_)
                    load_chunk(k)
                    xsc = [
                        xchs[k][0][:, dt, c * BH_ : (c + 1) * BH_] for c in (0, 1)
                    ]
                    rzm = ppool.tile([U, 2, 4 * BH_], F32, tag="rzm", name="rzm")
                    xh0 = ppool.tile([U, 4 * BH_], F32, tag="xh0", name="xh0")
                    xh1 = ppool.tile([U, 4 * BH_], F32, tag="xh1", name="xh1")
                    xhp = (xh0, xh1)
                    for c in (0, 1):
                        nc.tensor.matmul(
                            rzm[:, c, 0:BH_], w_r[:], xsc[c], start=True, stop=False
                        )
                    for c in (0, 1):
                        nc.tensor.matmul(
                            rzm[:, c, BH_ : 2 * BH_], w_z[:], xsc[c],
                            start=False, stop=False,
                        )
                    nc.tensor.matmul(
                        xh0[:, 0:BH_], w_h[:], xsc[0], start=True, stop=False
                    )
                    nc.tensor.matmul(
                        xh1[:, 0:BH_], w_h[:], xsc[1], start=True, stop=False
                    )
                    if has_bias:
                        # bias rows broadcast over batch: b_r -> r regions, etc.
                        for c in (0, 1):
                            nc.tensor.matmul(
                                rzm[:, c, 0:BH_], btile[0:1, :], onesr[:, 0:BH_],
                                start=False, stop=False,
                            )
                        for c in (0, 1):
                            nc.tensor.matmul(
                                rzm[:, c, BH_ : 2 * BH_], btile[1:2, :],
                                onesr[:, 0:BH_], start=False, stop=False,
                            )
                        nc.tensor.matmul(
                            xh0[:, 0:BH_], btile[2:3, :], onesr[:, 0:BH_],
                            start=False, stop=False,
                        )
                        nc.tensor.matmul(
                            xh1[:, 0:BH_], btile[2:3, :], onesr[:, 0:BH_],
                            start=False, stop=False,
                        )
                    return rzm, xhp

                def sig_phase(c, t, ps):
                    rzm, _xhp = ps
                    k, dt = divmod(t, TC_)
                    cs = slice(c * BH_, (c + 1) * BH_)
                    h = h_prev[c]
                    nc.tensor.matmul(
                        rzm[:, c, 0:BH_], u_r[:], h, start=False, stop=False
                    )
                    nc.tensor.matmul(
                        rzm[:, c, BH_ : 2 * BH_], u_z[:], h, start=False, stop=False
                    )
                    nc.tensor.matmul(
                        rzm[:, c, 2 * BH_ : 3 * BH_], u_h[:], h, start=False, stop=True
                    )
                    rz = wpool.tile([U, 2 * BH_], BF16, tag=f"rz{c}", name="rz")
                    nc.scalar.activation(rz[:], rzm[:, c, 0 : 2 * BH_], AF.Sigmoid)
                    t1 = wpool.tile([U, BH_], BF16, tag=f"t1{c}", name="t1")
                    nc.vector.tensor_tensor(
                        t1[:], rzm[:, c, 2 * BH_ : 3 * BH_], rz[:, 0:BH_], OP.mult
                    )
                    uhat = wpool.tile([U, BH_], BF16, tag=f"uhat{c}", name="uhat")
                    nc.gpsimd.tensor_tensor(
                        uhat[:], rz[:, BH_:], xchs[k][1][:, dt, cs], OP.mult
                    )
                    m1 = wpool.tile([U, BH_], BF16, tag=f"m1{c}", name="m1")
                    nc.vector.scalar_tensor_tensor(
                        m1[:], uhat[:], 1.0, h, OP.subtract, OP.mult
                    )
                    return t1, uhat, m1

                def tanh_phase(c, t, ps, t1, uhat, m1):
                    _rzm, xhp = ps
                    k, dt = divmod(t, TC_)
                    cs = slice(c * BH_, (c + 1) * BH_)
                    xh_c = xhp[c]
                    nc.tensor.matmul(
                        xh_c[:, 0:BH_], ident[:], t1[:], start=False, stop=True
                    )
                    htil = wpool.tile([U, BH_], BF16, tag=f"htil{c}", name="htil")
                    nc.scalar.activation(htil[:], xh_c[:, 0:BH_], AF.Tanh)
                    m2 = wpool.tile([U, BH_], BF16, tag=f"m2{c}", name="m2")
                    nc.vector.tensor_tensor(m2[:], uhat[:], htil[:], OP.mult)
                    hn = ochs[k][:, dt, cs]
                    nc.vector.tensor_tensor(hn, m2[:], m1[:], OP.subtract)
                    h_prev[c] = hn
                    if c == 1 and dt == TC_ - 1:
                        nc.sync.dma_start(
                            outt[:, k * TC_ : (k + 1) * TC_, :], ochs[k][:]
                        )
                        xchs.pop(k, None)

                load_chunk(0)
                ps_sig = xside(0)
                ps_all = {0: ps_sig, 1: ps_sig}
                pend = {}
                smid = {}
                ps_b = ps_sig
                for t in range(T_):
                    k, dt = divmod(t, TC_)
                    if dt == 0:
                        ochs[k] = opool.tile(
                            [U, TC_, BL_], BF16, tag="och", name="och"
                        )
                        load_chunk(k + 1)
                    # A sig (t)
                    pend[0] = sig_phase(0, t, ps_all[0])
                    ps_a = ps_all[0]
                    # x-side for t+1 (both chains)
                    if t + 1 < T_:
                        ps_nxt = xside(t + 1)
                    else:
                        ps_nxt = None
                    # B tanh (t-1)
                    if t > 0:
                        tanh_phase(1, t - 1, ps_b, *pend[1])
                    # A tanh (t)
                    tanh_phase(0, t, ps_a, *pend[0])
                    # B sig (t)
                    pend[1] = sig_phase(1, t, ps_all[1])
                    ps_b = ps_all[1]
                    if ps_nxt is not None:
                        ps_all = {0: ps_nxt, 1: ps_nxt}
                tanh_phase(1, T_ - 1, ps_b, *pend[1])

    nc.compile()
    return nc


def _prep_core(inputs_np, h0_np, c, T_=T, BL_=BL):
    bf = ml_dtypes.bfloat16
    sl = slice(c * BL_, (c + 1) * BL_)
    x = inputs_np[sl, :, :U]  # [BL, T, U]
    a = inputs_np[sl, :, U]  # [BL, T]
    xt_c = np.ascontiguousarray(x.transpose(2, 1, 0)).astype(bf)  # [U,T,BL]
    a_tb = np.ascontiguousarray(a.T).astype(bf)  # [T, BL]
    abc_c = np.ascontiguousarray(np.broadcast_to(a_tb[None], (U, T_, BL_)))
    h0t_c = np.ascontiguousarray(h0_np[sl].T).astype(bf)  # [U, BL]
    return xt_c, abc_c, h0t_c


def _wcat(ws):
    bf = ml_dtypes.bfloat16
    W_r, U_r, W_z, U_z, W_h, U_h = ws
    return np.stack(
        [w.astype(bf) for w in (W_r, U_r, W_z, U_z, W_h, U_h, -U_r, -U_z, -U_h)]
    )


def kernel(inputs, h0, W_r, U_r, b_r, W_z, U_z, b_z, W_h, U_h, b_h):
    global LAST_RESULT, LAST_IN_MAPS
    bf = ml_dtypes.bfloat16
    inputs = np.asarray(inputs, dtype=np.float32)
    h0 = np.asarray(h0, dtype=np.float32)
    ws = [np.asarray(w, dtype=np.float32) for w in (W_r, U_r, W_z, U_z, W_h, U_h)]
    bs = [np.asarray(b, dtype=np.float32) for b in (b_r, b_z, b_h)]

    has_bias = bool(any(np.any(b) for b in bs))
    if has_bias not in _cache:
        _cache[has_bias] = _build(has_bias)
    nc = _cache[has_bias]

    wcat = _wcat(ws)
    brow = np.stack(bs).astype(bf)  # [3, U]

    in_maps = []
    for c in range(NCORES):
        xt_c, abc_c, h0t_c = _prep_core(inputs, h0, c)
        in_maps.append(
            {
                "xt": xt_c,
                "abc": abc_c,
                "h0t": h0t_c,
                "wcat": wcat,
                "brow": brow,
                "ident": np.eye(U, dtype=bf),
            }
        )

    res = run_bass_kernel_spmd(nc, in_maps, list(range(NCORES)), trace=PROFILE)
    LAST_IN_MAPS = in_maps
    LAST_RESULT = res

    out = np.empty((B, T, U), dtype=np.float32)
    for c in range(NCORES):
        sl = slice(c * BL, (c + 1) * BL)
        out[sl] = res.results[c]["outt"].astype(np.float32).transpose(2, 1, 0)
    return out


# revision 3
# speedup vs baseline: 1.6530x; 1.2347x over previous
"""Custom GRU cell kernel v4: two interleaved batch-chains, weight-switch-aware.

HW-calibrated cost model (measured): a matmul whose stationary weight differs
from the previous one pays ~+95ns for the weight load; same-weight is free.
ACT activation = (FD+222)*0.833ns, DVE TT = (FD/2+58)*1.042 (bf16 SBUF) or
(FD+120)*1.042 (PSUM src), Pool TT = ~165ns @ FD=128.

Design:
- chains A (batch cols 0:128) and B (128:256), independent recurrences,
  phases interleaved for latency hiding (as v3).
- x-side projections for both chains done as single FD=256 matmuls writing
  both chains' PSUM banks through one [U, 2, 512] two-bank tile (strided
  output AP) -> 3 weight switches per step instead of 6.
- no identity matmul: s = xh + t1 computed on DVE (PSUM + SBUF add); tanh
  reads s from SBUF. The xh bank closes at x-side time.
- recurrent matmuls per chain FD=128 (3 switches per chain).
- elementwise: DVE does t1, s, m1(STT); GPSIMD does uhat, m2, hn.

PSUM per step: rzm_both [U,2,512] = [r|z|mmh|-]*2 chains (bank c closes at
chain c's uh matmul), xhp [U,2,256] = both xh (closes at wh x-pair).
"""

import sys

sys.path.insert(0, "/opt/trn_rl_repo")

import numpy as np
import ml_dtypes

import concourse.bass as bass  # noqa: F401
import concourse.mybir as mybir
import concourse.tile as tile
from concourse import bacc
from concourse.bass_utils import run_bass_kernel_spmd

BF16 = mybir.dt.bfloat16
F32 = mybir.dt.float32
AF = mybir.ActivationFunctionType
OP = mybir.AluOpType

B, T, U = 2048, 200, 128
NCORES = 8
BL = B // NCORES  # 256
BH = BL // 2  # 128 per chain
TC = 25
NCHUNK = T // TC

PROFILE = False
LAST_RESULT = None
LAST_IN_MAPS = None

_cache = {}


def _build(has_bias: bool, T_=T, TC_=TC, BL_=BL, reps=1):
    BH_ = BL_ // 2
    NCHUNK_ = T_ // TC_
    nc = bacc.Bacc("TRN2", target_bir_lowering=False)

    xt = nc.dram_tensor("xt", [U, T_, BL_], BF16, kind="ExternalInput")
    abc = nc.dram_tensor("abc", [U, T_, BL_], BF16, kind="ExternalInput")
    h0t = nc.dram_tensor("h0t", [U, BL_], BF16, kind="ExternalInput")
    wcat = nc.dram_tensor("wcat", [9, U, U], BF16, kind="ExternalInput")
    ident_d = nc.dram_tensor("ident", [U, U], BF16, kind="ExternalInput")
    brow_d = nc.dram_tensor("brow", [3, U], BF16, kind="ExternalInput")
    outt = nc.dram_tensor("outt", [U, T_, BL_], BF16, kind="ExternalOutput")

    with tile.TileContext(nc) as tc:
        with (
            tc.tile_pool(name="const", bufs=1) as cpool,
            tc.tile_pool(name="xchunk", bufs=2) as xpool,
            tc.tile_pool(name="achunk", bufs=2) as apool,
            tc.tile_pool(name="ochunk", bufs=2) as opool,
            tc.tile_pool(name="work", bufs=3) as wpool,
            tc.tile_pool(name="psum", bufs=2, space="PSUM") as ppool,
        ):
            wts = []
            for i in range(6):
                wt = cpool.tile([U, U], BF16, tag=f"w{i}", name=f"w{i}")
                nc.sync.dma_start(wt[:], wcat[i])
                wts.append(wt)
            w_r, u_r, w_z, u_z, w_h, u_h = wts
            ident = cpool.tile([U, U], BF16, tag="ident")
            nc.sync.dma_start(ident[:], ident_d[:])
            h0tile = cpool.tile([U, BL_], BF16, tag="h0")
            nc.sync.dma_start(h0tile[:], h0t[:])
            if has_bias:
                btile = cpool.tile([3, U], BF16, tag="brow")
                nc.sync.dma_start(btile[:], brow_d[:])
                onesr = cpool.tile([1, 2 * BH_], BF16, tag="onesr")
                nc.vector.memset(onesr[:], 1.0)

            for _rep in range(reps):
                xchs = {}
                ochs = {}
                h_prev = {0: h0tile[:, 0:BH_], 1: h0tile[:, BH_:]}

                def load_chunk(k):
                    if k >= NCHUNK_ or k in xchs:
                        return
                    t0, t1x = k * TC_, (k + 1) * TC_
                    xch = xpool.tile([U, TC_, BL_], BF16, tag="xch", name="xch")
                    nc.sync.dma_start(xch[:], xt[:, t0:t1x, :])
                    ach = apool.tile([U, TC_, BL_], BF16, tag="ach", name="ach")
                    nc.sync.dma_start(ach[:], abc[:, t0:t1x, :])
                    xchs[k] = (xch, ach)

                def xside(t):
                    """x-projections for BOTH chains, step t. Matmuls ordered in
                    same-weight runs (second of each pair reloads no weights).
                    Per-matmul outputs stay within one PSUM bank."""
                    k, dt = divmod(t, TC_)
                    load_chunk(k)
                    xsc = [
                        xchs[k][0][:, dt, c * BH_ : (c + 1) * BH_] for c in (0, 1)
                    ]
                    rzm = ppool.tile([U, 2, 4 * BH_], F32, tag="rzm", name="rzm")
                    xh0 = ppool.tile([U, 4 * BH_], F32, tag="xh0", name="xh0")
                    xh1 = ppool.tile([U, 4 * BH_], F32, tag="xh1", name="xh1")
                    xhp = (xh0, xh1)
                    for c in (0, 1):
                        nc.tensor.matmul(
                            rzm[:, c, 0:BH_], w_r[:], xsc[c], start=True, stop=False
                        )
                    for c in (0, 1):
                        nc.tensor.matmul(
                            rzm[:, c, BH_ : 2 * BH_], w_z[:], xsc[c],
                            start=False, stop=False,
                        )
                    nc.tensor.matmul(
                        xh0[:, 0:BH_], w_h[:], xsc[0], start=True, stop=False
                    )
                    nc.tensor.matmul(
                        xh1[:, 0:BH_], w_h[:], xsc[1], start=True, stop=False
                    )
                    if has_bias:
                        # bias rows broadcast over batch: b_r -> r regions, etc.
                        for c in (0, 1):
                            nc.tensor.matmul(
                                rzm[:, c, 0:BH_], btile[0:1, :], onesr[:, 0:BH_],
                                start=False, stop=False,
                            )
                        for c in (0, 1):
                            nc.tensor.matmul(
                                rzm[:, c, BH_ : 2 * BH_], btile[1:2, :],
                                onesr[:, 0:BH_], start=False, stop=False,
                            )
                        nc.tensor.matmul(
                            xh0[:, 0:BH_], btile[2:3, :], onesr[:, 0:BH_],
                            start=False, stop=False,
                        )
                        nc.tensor.matmul(
                            xh1[:, 0:BH_], btile[2:3, :], onesr[:, 0:BH_],
                            start=False, stop=False,
                        )
                    return rzm, xhp

                def sig_phase(c, t, ps):
                    rzm, _xhp = ps
                    k, dt = divmod(t, TC_)
                    cs = slice(c * BH_, (c + 1) * BH_)
                    h = h_prev[c]
                    # chain A runs uh first / chain B runs uh last so that the
                    # B->A boundary across iterations reuses the loaded U_h
                    if c == 0:
                        nc.tensor.matmul(
                            rzm[:, c, 2 * BH_ : 3 * BH_], u_h[:], h,
                            start=False, stop=False,
                        )
                        nc.tensor.matmul(
                            rzm[:, c, BH_ : 2 * BH_], u_z[:], h,
                            start=False, stop=False,
                        )
                        nc.tensor.matmul(
                            rzm[:, c, 0:BH_], u_r[:], h, start=False, stop=True
                        )
                    else:
                        nc.tensor.matmul(
                            rzm[:, c, 0:BH_], u_r[:], h, start=False, stop=False
                        )
                        nc.tensor.matmul(
                            rzm[:, c, BH_ : 2 * BH_], u_z[:], h,
                            start=False, stop=False,
                        )
                        nc.tensor.matmul(
                            rzm[:, c, 2 * BH_ : 3 * BH_], u_h[:], h,
                            start=False, stop=True,
                        )
                    rz = wpool.tile([U, 2 * BH_], BF16, tag=f"rz{c}", name="rz")
                    nc.scalar.activation(rz[:], rzm[:, c, 0 : 2 * BH_], AF.Sigmoid)
                    t1 = wpool.tile([U, BH_], BF16, tag=f"t1{c}", name="t1")
                    nc.vector.tensor_tensor(
                        t1[:], rzm[:, c, 2 * BH_ : 3 * BH_], rz[:, 0:BH_], OP.mult
                    )
                    uhat = wpool.tile([U, BH_], BF16, tag=f"uhat{c}", name="uhat")
                    nc.gpsimd.tensor_tensor(
                        uhat[:], rz[:, BH_:], xchs[k][1][:, dt, cs], OP.mult
                    )
                    m1 = wpool.tile([U, BH_], BF16, tag=f"m1{c}", name="m1")
                    nc.vector.scalar_tensor_tensor(
                        m1[:], uhat[:], 1.0, h, OP.subtract, OP.mult
                    )
                    return t1, uhat, m1

                def tanh_phase(c, t, ps, t1, uhat, m1):
                    _rzm, xhp = ps
                    k, dt = divmod(t, TC_)
                    cs = slice(c * BH_, (c + 1) * BH_)
                    xh_c = xhp[c]
                    nc.tensor.matmul(
                        xh_c[:, 0:BH_], ident[:], t1[:], start=False, stop=True
                    )
                    htil = wpool.tile([U, BH_], BF16, tag=f"htil{c}", name="htil")
                    nc.scalar.activation(htil[:], xh_c[:, 0:BH_], AF.Tanh)
                    m2 = wpool.tile([U, BH_], BF16, tag=f"m2{c}", name="m2")
                    nc.vector.tensor_tensor(m2[:], uhat[:], htil[:], OP.mult)
                    hn = ochs[k][:, dt, cs]
                    nc.vector.tensor_tensor(hn, m2[:], m1[:], OP.subtract)
                    h_prev[c] = hn
                    if c == 1 and dt == TC_ - 1:
                        nc.sync.dma_start(
                            outt[:, k * TC_ : (k + 1) * TC_, :], ochs[k][:]
                        )
                        xchs.pop(k, None)

                load_chunk(0)
                ps_sig = xside(0)
                ps_all = {0: ps_sig, 1: ps_sig}
                pend = {}
                smid = {}
                ps_b = ps_sig
                for t in range(T_):
                    k, dt = divmod(t, TC_)
                    if dt == 0:
                        ochs[k] = opool.tile(
                            [U, TC_, BL_], BF16, tag="och", name="och"
                        )
                        load_chunk(k + 1)
                    # A sig (t)
                    pend[0] = sig_phase(0, t, ps_all[0])
                    ps_a = ps_all[0]
                    # x-side for t+1 (both chains)
                    if t + 1 < T_:
                        ps_nxt = xside(t + 1)
                    else:
                        ps_nxt = None
                    # B tanh (t-1)
                    if t > 0:
                        tanh_phase(1, t - 1, ps_b, *pend[1])
                    # A tanh (t)
                    tanh_phase(0, t, ps_a, *pend[0])
                    # B sig (t)
                    pend[1] = sig_phase(1, t, ps_all[1])
                    ps_b = ps_all[1]
                    if ps_nxt is not None:
                        ps_all = {0: ps_nxt, 1: ps_nxt}
                tanh_phase(1, T_ - 1, ps_b, *pend[1])

    nc.compile()
    return nc


def _prep_core(inputs_np, h0_np, c, T_=T, BL_=BL):
    bf = ml_dtypes.bfloat16
    sl = slice(c * BL_, (c + 1) * BL_)
    x = inputs_np[sl, :, :U]  # [BL, T, U]
    a = inputs_np[sl, :, U]  # [BL, T]
    xt_c = np.ascontiguousarray(x.transpose(2, 1, 0)).astype(bf)  # [U,T,BL]
    a_tb = np.ascontiguousarray(a.T).astype(bf)  # [T, BL]
    abc_c = np.ascontiguousarray(np.broadcast_to(a_tb[None], (U, T_, BL_)))
    h0t_c = np.ascontiguousarray(h0_np[sl].T).astype(bf)  # [U, BL]
    return xt_c, abc_c, h0t_c


def _wcat(ws):
    bf = ml_dtypes.bfloat16
    W_r, U_r, W_z, U_z, W_h, U_h = ws
    return np.stack(
        [w.astype(bf) for w in (W_r, U_r, W_z, U_z, W_h, U_h, -U_r, -U_z, -U_h)]
    )


def kernel(inputs, h0, W_r, U_r, b_r, W_z, U_z, b_z, W_h, U_h, b_h):
    global LAST_RESULT, LAST_IN_MAPS
    bf = ml_dtypes.bfloat16
    inputs = np.asarray(inputs, dtype=np.float32)
    h0 = np.asarray(h0, dtype=np.float32)
    ws = [np.asarray(w, dtype=np.float32) for w in (W_r, U_r, W_z, U_z, W_h, U_h)]
    bs = [np.asarray(b, dtype=np.float32) for b in (b_r, b_z, b_h)]

    has_bias = bool(any(np.any(b) for b in bs))
    if has_bias not in _cache:
        _cache[has_bias] = _build(has_bias)
    nc = _cache[has_bias]

    wcat = _wcat(ws)
    brow = np.stack(bs).astype(bf)  # [3, U]

    in_maps = []
    for c in range(NCORES):
        xt_c, abc_c, h0t_c = _prep_core(inputs, h0, c)
        in_maps.append(
            {
                "xt": xt_c,
                "abc": abc_c,
                "h0t": h0t_c,
                "wcat": wcat,
                "brow": brow,
                "ident": np.eye(U, dtype=bf),
            }
        )

    res = run_bass_kernel_spmd(nc, in_maps, list(range(NCORES)), trace=PROFILE)
    LAST_IN_MAPS = in_maps
    LAST_RESULT = res

    out = np.empty((B, T, U), dtype=np.float32)
    for c in range(NCORES):
        sl = slice(c * BL, (c + 1) * BL)
        out[sl] = res.results[c]["outt"].astype(np.float32).transpose(2, 1, 0)
    return out


# revision 4
# speedup vs baseline: 1.8374x; 1.1115x over previous
"""Custom GRU cell kernel v4: two interleaved batch-chains, weight-switch-aware.

HW-calibrated cost model (measured): a matmul whose stationary weight differs
from the previous one pays ~+95ns for the weight load; same-weight is free.
ACT activation = (FD+222)*0.833ns, DVE TT = (FD/2+58)*1.042 (bf16 SBUF) or
(FD+120)*1.042 (PSUM src), Pool TT = ~165ns @ FD=128.

Design:
- chains A (batch cols 0:128) and B (128:256), independent recurrences,
  phases interleaved for latency hiding (as v3).
- x-side projections for both chains done as single FD=256 matmuls writing
  both chains' PSUM banks through one [U, 2, 512] two-bank tile (strided
  output AP) -> 3 weight switches per step instead of 6.
- no identity matmul: s = xh + t1 computed on DVE (PSUM + SBUF add); tanh
  reads s from SBUF. The xh bank closes at x-side time.
- recurrent matmuls per chain FD=128 (3 switches per chain).
- elementwise: DVE does t1, s, m1(STT); GPSIMD does uhat, m2, hn.

PSUM per step: rzm_both [U,2,512] = [r|z|mmh|-]*2 chains (bank c closes at
chain c's uh matmul), xhp [U,2,256] = both xh (closes at wh x-pair).
"""

import sys

sys.path.insert(0, "/opt/trn_rl_repo")

import numpy as np
import ml_dtypes

import concourse.bass as bass  # noqa: F401
import concourse.mybir as mybir
import concourse.tile as tile
from concourse import bacc
from concourse.bass_utils import run_bass_kernel_spmd

BF16 = mybir.dt.bfloat16
F32 = mybir.dt.float32
AF = mybir.ActivationFunctionType
OP = mybir.AluOpType

B, T, U = 2048, 200, 128
NCORES = 8
BL = B // NCORES  # 256
BH = BL // 2  # 128 per chain
TC = 25
NCHUNK = T // TC

PROFILE = False
LAST_RESULT = None
LAST_IN_MAPS = None

_cache = {}


def _build(has_bias: bool, T_=T, TC_=TC, BL_=BL, reps=1):
    BH_ = BL_ // 2
    NCHUNK_ = T_ // TC_
    nc = bacc.Bacc("TRN2", target_bir_lowering=False)

    xt = nc.dram_tensor("xt", [U, T_, BL_], BF16, kind="ExternalInput")
    abc = nc.dram_tensor("abc", [U, T_, BL_], BF16, kind="ExternalInput")
    h0t = nc.dram_tensor("h0t", [U, BL_], BF16, kind="ExternalInput")
    wcat = nc.dram_tensor("wcat", [9, U, U], BF16, kind="ExternalInput")
    ident_d = nc.dram_tensor("ident", [U, U], BF16, kind="ExternalInput")
    brow_d = nc.dram_tensor("brow", [3, U], BF16, kind="ExternalInput")
    outt = nc.dram_tensor("outt", [U, T_, BL_], BF16, kind="ExternalOutput")

    with tile.TileContext(nc) as tc:
        with (
            tc.tile_pool(name="const", bufs=1) as cpool,
            tc.tile_pool(name="xchunk", bufs=2) as xpool,
            tc.tile_pool(name="achunk", bufs=2) as apool,
            tc.tile_pool(name="ochunk", bufs=2) as opool,
            tc.tile_pool(name="work", bufs=3) as wpool,
            tc.tile_pool(name="psum", bufs=2, space="PSUM") as ppool,
        ):
            wts = []
            for i in range(6):
                wt = cpool.tile([U, U], BF16, tag=f"w{i}", name=f"w{i}")
                nc.sync.dma_start(wt[:], wcat[i])
                wts.append(wt)
            w_r, u_r, w_z, u_z, w_h, u_h = wts
            ident = cpool.tile([U, U], BF16, tag="ident")
            nc.sync.dma_start(ident[:], ident_d[:])
            h0tile = cpool.tile([U, BL_], BF16, tag="h0")
            nc.sync.dma_start(h0tile[:], h0t[:])
            if has_bias:
                btile = cpool.tile([3, U], BF16, tag="brow")
                nc.sync.dma_start(btile[:], brow_d[:])
                onesr = cpool.tile([1, 2 * BH_], BF16, tag="onesr")
                nc.vector.memset(onesr[:], 1.0)

            for _rep in range(reps):
                xchs = {}
                ochs = {}
                h_prev = {0: h0tile[:, 0:BH_], 1: h0tile[:, BH_:]}

                def load_chunk(k):
                    if k >= NCHUNK_ or k in xchs:
                        return
                    t0, t1x = k * TC_, (k + 1) * TC_
                    xch = xpool.tile([U, TC_, BL_], BF16, tag="xch", name="xch")
                    nc.sync.dma_start(xch[:], xt[:, t0:t1x, :])
                    ach = apool.tile([U, TC_, BL_], BF16, tag="ach", name="ach")
                    nc.sync.dma_start(ach[:], abc[:, t0:t1x, :])
                    xchs[k] = (xch, ach)

                def xside(t):
                    """x-projections for BOTH chains, step t. Matmuls ordered in
                    same-weight runs (second of each pair reloads no weights).
                    Per-matmul outputs stay within one PSUM bank."""
                    k, dt = divmod(t, TC_)
                    load_chunk(k)
                    xsc = [
                        xchs[k][0][:, dt, c * BH_ : (c + 1) * BH_] for c in (0, 1)
                    ]
                    rzm0 = ppool.tile([U, 4 * BH_], F32, tag="rzm0", name="rzm0")
                    rzm1 = ppool.tile([U, 4 * BH_], F32, tag="rzm1", name="rzm1")
                    rzm = (rzm0, rzm1)
                    xh0 = ppool.tile([U, 4 * BH_], F32, tag="xh0", name="xh0")
                    xh1 = ppool.tile([U, 4 * BH_], F32, tag="xh1", name="xh1")
                    xhp = (xh0, xh1)
                    for c in (0, 1):
                        nc.tensor.matmul(
                            rzm[c][:, 0:BH_], w_r[:], xsc[c], start=True, stop=False
                        )
                    for c in (0, 1):
                        nc.tensor.matmul(
                            rzm[c][:, BH_ : 2 * BH_], w_z[:], xsc[c],
                            start=False, stop=False,
                        )
                    nc.tensor.matmul(
                        xh0[:, 0:BH_], w_h[:], xsc[0], start=True, stop=False
                    )
                    nc.tensor.matmul(
                        xh1[:, 0:BH_], w_h[:], xsc[1], start=True, stop=False
                    )
                    if has_bias:
                        # bias rows broadcast over batch: b_r -> r regions, etc.
                        for c in (0, 1):
                            nc.tensor.matmul(
                                rzm[c][:, 0:BH_], btile[0:1, :], onesr[:, 0:BH_],
                                start=False, stop=False,
                            )
                        for c in (0, 1):
                            nc.tensor.matmul(
                                rzm[c][:, BH_ : 2 * BH_], btile[1:2, :],
                                onesr[:, 0:BH_], start=False, stop=False,
                            )
                        nc.tensor.matmul(
                            xh0[:, 0:BH_], btile[2:3, :], onesr[:, 0:BH_],
                            start=False, stop=False,
                        )
                        nc.tensor.matmul(
                            xh1[:, 0:BH_], btile[2:3, :], onesr[:, 0:BH_],
                            start=False, stop=False,
                        )
                    return rzm, xhp

                def sig_phase(c, t, ps):
                    rzmp, _xhp = ps
                    rzmc = rzmp[c]
                    k, dt = divmod(t, TC_)
                    cs = slice(c * BH_, (c + 1) * BH_)
                    h = h_prev[c]
                    # chain A runs uh first / chain B runs uh last so that the
                    # B->A boundary across iterations reuses the loaded U_h
                    if c == 0:
                        nc.tensor.matmul(
                            rzmc[:, 2 * BH_ : 3 * BH_], u_h[:], h,
                            start=False, stop=False,
                        )
                        nc.tensor.matmul(
                            rzmc[:, BH_ : 2 * BH_], u_z[:], h,
                            start=False, stop=False,
                        )
                        nc.tensor.matmul(
                            rzmc[:, 0:BH_], u_r[:], h, start=False, stop=True
                        )
                    else:
                        nc.tensor.matmul(
                            rzmc[:, 0:BH_], u_r[:], h, start=False, stop=False
                        )
                        nc.tensor.matmul(
                            rzmc[:, BH_ : 2 * BH_], u_z[:], h,
                            start=False, stop=False,
                        )
                        nc.tensor.matmul(
                            rzmc[:, 2 * BH_ : 3 * BH_], u_h[:], h,
                            start=False, stop=True,
                        )
                    rz = wpool.tile([U, 2 * BH_], BF16, tag=f"rz{c}", name="rz")
                    nc.scalar.activation(rz[:], rzmc[:, 0 : 2 * BH_], AF.Sigmoid)
                    t1 = wpool.tile([U, BH_], BF16, tag=f"t1{c}", name="t1")
                    nc.vector.tensor_tensor(
                        t1[:], rzmc[:, 2 * BH_ : 3 * BH_], rz[:, 0:BH_], OP.mult
                    )
                    uhat = wpool.tile([U, BH_], BF16, tag=f"uhat{c}", name="uhat")
                    nc.gpsimd.tensor_tensor(
                        uhat[:], rz[:, BH_:], xchs[k][1][:, dt, cs], OP.mult
                    )
                    m1 = wpool.tile([U, BH_], BF16, tag=f"m1{c}", name="m1")
                    nc.vector.scalar_tensor_tensor(
                        m1[:], uhat[:], 1.0, h, OP.subtract, OP.mult
                    )
                    return t1, uhat, m1

                def tanh_phase(c, t, ps, t1, uhat, m1):
                    _rzm, xhp = ps
                    k, dt = divmod(t, TC_)
                    cs = slice(c * BH_, (c + 1) * BH_)
                    xh_c = xhp[c]
                    nc.tensor.matmul(
                        xh_c[:, 0:BH_], ident[:], t1[:], start=False, stop=True
                    )
                    htil = wpool.tile([U, BH_], BF16, tag=f"htil{c}", name="htil")
                    nc.scalar.activation(htil[:], xh_c[:, 0:BH_], AF.Tanh)
                    m2 = wpool.tile([U, BH_], BF16, tag=f"m2{c}", name="m2")
                    nc.vector.tensor_tensor(m2[:], uhat[:], htil[:], OP.mult)
                    hn = ochs[k][:, dt, cs]
                    nc.vector.tensor_tensor(hn, m2[:], m1[:], OP.subtract)
                    h_prev[c] = hn
                    if c == 1 and dt == TC_ - 1:
                        nc.sync.dma_start(
                            outt[:, k * TC_ : (k + 1) * TC_, :], ochs[k][:]
                        )
                        xchs.pop(k, None)

                load_chunk(0)
                ps_sig = xside(0)
                ps_all = {0: ps_sig, 1: ps_sig}
                pend = {}
                smid = {}
                ps_b = ps_sig
                for t in range(T_):
                    k, dt = divmod(t, TC_)
                    if dt == 0:
                        ochs[k] = opool.tile(
                            [U, TC_, BL_], BF16, tag="och", name="och"
                        )
                        load_chunk(k + 1)
                    # A sig (t)
                    pend[0] = sig_phase(0, t, ps_all[0])
                    ps_a = ps_all[0]
                    # x-side for t+1 (both chains)
                    if t + 1 < T_:
                        ps_nxt = xside(t + 1)
                    else:
                        ps_nxt = None
                    # B tanh (t-1)
                    if t > 0:
                        tanh_phase(1, t - 1, ps_b, *pend[1])
                    # A tanh (t)
                    tanh_phase(0, t, ps_a, *pend[0])
                    # B sig (t)
                    pend[1] = sig_phase(1, t, ps_all[1])
                    ps_b = ps_all[1]
                    if ps_nxt is not None:
                        ps_all = {0: ps_nxt, 1: ps_nxt}
                tanh_phase(1, T_ - 1, ps_b, *pend[1])

    nc.compile()
    return nc


def _prep_core(inputs_np, h0_np, c, T_=T, BL_=BL):
    bf = ml_dtypes.bfloat16
    sl = slice(c * BL_, (c + 1) * BL_)
    x = inputs_np[sl, :, :U]  # [BL, T, U]
    a = inputs_np[sl, :, U]  # [BL, T]
    xt_c = np.ascontiguousarray(x.transpose(2, 1, 0)).astype(bf)  # [U,T,BL]
    a_tb = np.ascontiguousarray(a.T).astype(bf)  # [T, BL]
    abc_c = np.ascontiguousarray(np.broadcast_to(a_tb[None], (U, T_, BL_)))
    h0t_c = np.ascontiguousarray(h0_np[sl].T).astype(bf)  # [U, BL]
    return xt_c, abc_c, h0t_c


def _wcat(ws):
    bf = ml_dtypes.bfloat16
    W_r, U_r, W_z, U_z, W_h, U_h = ws
    return np.stack(
        [w.astype(bf) for w in (W_r, U_r, W_z, U_z, W_h, U_h, -U_r, -U_z, -U_h)]
    )


def kernel(inputs, h0, W_r, U_r, b_r, W_z, U_z, b_z, W_h, U_h, b_h):
    global LAST_RESULT, LAST_IN_MAPS
    bf = ml_dtypes.bfloat16
    inputs = np.asarray(inputs, dtype=np.float32)
    h0 = np.asarray(h0, dtype=np.float32)
    ws = [np.asarray(w, dtype=np.float32) for w in (W_r, U_r, W_z, U_z, W_h, U_h)]
    bs = [np.asarray(b, dtype=np.float32) for b in (b_r, b_z, b_h)]

    has_bias = bool(any(np.any(b) for b in bs))
    if has_bias not in _cache:
        _cache[has_bias] = _build(has_bias)
    nc = _cache[has_bias]

    wcat = _wcat(ws)
    brow = np.stack(bs).astype(bf)  # [3, U]

    in_maps = []
    for c in range(NCORES):
        xt_c, abc_c, h0t_c = _prep_core(inputs, h0, c)
        in_maps.append(
            {
                "xt": xt_c,
                "abc": abc_c,
                "h0t": h0t_c,
                "wcat": wcat,
                "brow": brow,
                "ident": np.eye(U, dtype=bf),
            }
        )

    res = run_bass_kernel_spmd(nc, in_maps, list(range(NCORES)), trace=PROFILE)
    LAST_IN_MAPS = in_maps
    LAST_RESULT = res

    out = np.empty((B, T, U), dtype=np.float32)
    for c in range(NCORES):
        sl = slice(c * BL, (c + 1) * BL)
        out[sl] = res.results[c]["outt"].astype(np.float32).transpose(2, 1, 0)
    return out
